# revision 35
# baseline (speedup 1.0000x reference)
"""Causal single-head attention (B=4, S=2048, d=1024, f32) on 8 TRN2 NeuronCores.

Sharding: core i = (batch b = i//2, half h = i%2). Queries are assigned
zig-zag over 256-row blocks (h=0 gets 0,2,5,7; h=1 gets 1,3,4,6) and each
core processes four 256-query chunks against KV prefixes of 512/1024/1536/
2048 keys. Exact causality via host-precomputed additive masks (0 / -1e30)
added to score PSUM before exp.

K/V projections are deduplicated across the two cores of a batch: each core
computes K^T and V only for the 256-row blocks of its own parity (h=0 even
blocks, h=1 odd), then the pair exchanges halves with an HBM AllGather over
replica groups [[0,1],[2,3],[4,5],[6,7]]. SPMD uniformity is preserved by
data placement: the host hands each core x^T with ITS parity's columns
packed into [0:1024), so the (identical) program always projects columns
[0:1024) and always scatters AllGather slot s into the true positions of
parity s. This cuts per-core projection work from 5.4 to 3.2 GMAC; the
exchange (2 MB out, 4 MB back per tensor) overlaps with the V/Q projections
on the PE.

Compute is bf16 on the TensorEngine with f32 PSUM accumulation; matmuls are
[c=128, m=128, n<=512]. Scores are computed transposed (S^T[k, q]) so P^T =
exp(S^T) feeds the AV matmul directly as lhsT, with the softmax denominator
from a ones-column matmul and the division folded into the PSUM->SBUF copy
of the output. No max-subtraction: scaled logits are bounded for these
inputs.

Hardware-wait notes: walrus accepts a single sync wait per engine
instruction (bacc legalizes the rest, at a cost). Every DMA lands in a
fresh/stable region and is "blessed" by an in-place DVE copy (uint32 view)
so matmul dependencies collapse onto the DVE semaphore; exp output is
DVE-copied into P^T for the same reason; PSUM lives in one pool (tags
big/rs = 6+2 = 8 banks) so slot WARs stay on DVE/ACT.

The `reps` parameter repeats the whole body inside the NEFF; test.py times
the 9x-vs-17x wall-clock slope (paired, alternating bursts) to estimate
per-execution device time robustly against axon dispatch jitter.
"""

import numpy as np
import ml_dtypes

import concourse.bass as bass
from concourse import bacc
import concourse.mybir as mybir
from concourse.tile import TileContext
from concourse.bass_utils import run_bass_kernel_spmd

P = 128
B = 4
S = 2048          # sequence length (= keys per batch)
D = 1024          # d_in = d_out
HALF = 1024       # queries per core; also K/V rows computed per core
CHUNK = 256       # query chunk
CD = D // P       # 8 contraction tiles
SK = S // P       # 16 key tiles
F = 512           # matmul moving free dim (one PSUM bank of f32)
KV = (512, 1024, 1536, 2048)   # kv prefix length per chunk
QBASE = (0, 256, 512, 768)     # local query offset of each chunk
SCALE = 1.0 / 32.0    # 1/sqrt(d_k)
NEG = -1.0e30         # additive mask for disallowed (k, q)

# global query-row block starts per half (zig-zag over 256-blocks)
QROWS = ((0, 512, 1280, 1792), (256, 768, 1024, 1536))

GROUPS = [[0, 1], [2, 3], [4, 5], [6, 7]]

BF16 = ml_dtypes.bfloat16


def build_nc(reps: int = 1) -> bacc.Bacc:
    nc = bacc.Bacc("TRN2")
    bf = mybir.dt.bfloat16
    f32 = mybir.dt.float32
    u32 = mybir.dt.uint32

    # x^T with this core's parity columns packed into [0:1024)
    xkv_d = nc.declare_dram_parameter("xkv", [D, HALF], bf, isOutput=False)
    xq_d = nc.declare_dram_parameter("xq", [D, HALF], bf, isOutput=False)
    wq_d = nc.declare_dram_parameter("wq", [D, D], bf, isOutput=False)
    wk_d = nc.declare_dram_parameter("wk", [D, D], bf, isOutput=False)
    wv_d = nc.declare_dram_parameter("wv", [D, D], bf, isOutput=False)
    m_d = [
        nc.declare_dram_parameter(f"mask{ci}", [KV[ci], CHUNK], bf, isOutput=False)
        for ci in range(len(KV))
    ]
    out_d = nc.declare_dram_parameter("out", [HALF, D], bf, isOutput=True)

    with TileContext(nc) as tc:
        with tc.tile_pool(name="persist", bufs=1) as persist, \
             tc.tile_pool(name="work", bufs=1) as work, \
             tc.tile_pool(name="dram", bufs=1, space="DRAM") as dram, \
             tc.tile_pool(name="psum", bufs=1, space="PSUM") as psum:
            # K^T[d, k] as [p, c, jj, 512] (true col = 512*jj + inner);
            # V[k, d] as [p, j, t, d] (true s-tile = 4*j + t);
            # Q^T[d, q] resident in SBUF (bf16)
            KT = persist.tile([P, CD, 4, F], bf)
            Vt = persist.tile([P, 4, 4, D], bf)
            QT = persist.tile([P, CD, HALF], bf)
            ones = persist.tile([P, 1], bf)
            nc.vector.memset(ones[:], 1.0)

            def load(dst, dram_t, c, eng=None):
                (eng or nc.sync).dma_start(out=dst[:, c],
                                           in_=dram_t[c * P:(c + 1) * P, :])
                v = dst[:, c].bitcast(u32)
                nc.vector.tensor_copy(v, v)

            for _rep in range(reps):
                # exchange bounce buffers (fresh per rep)
                kx_in = dram.tile([CD, P, HALF], bf, tag="kxi", name="kx_in")
                kx_out = dram.tile([2, CD, P, 4, CHUNK], bf, tag="kxo",
                                   name="kx_out")
                v_in = dram.tile([8, P, D], bf, tag="vxi", name="v_in")
                v_out = [dram.tile([2, 4, P, D], bf, tag=f"vxo{g}",
                                   name=f"v_out{g}") for g in range(2)]

                xkv_s = work.tile([P, CD, HALF], bf, tag="xkv")
                xq_s = work.tile([P, CD, HALF], bf, tag="xq")
                wq_s = work.tile([P, CD, D], bf, tag="wq")
                wk_s = work.tile([P, CD, D], bf, tag="wk")
                wv_s = work.tile([P, CD, D], bf, tag="wv")
                # wk/xkv then wv pace the K/V projections on the sync HWDGE;
                # wq/xq ride the second (Activation) HWDGE queue in parallel.
                for c in range(CD):
                    load(wk_s, wk_d, c)
                    load(xkv_s, xkv_d, c)
                    load(wq_s, wq_d, c, eng=nc.scalar)
                    load(xq_s, xq_d, c, eng=nc.scalar)
                for c in range(CD):
                    load(wv_s, wv_d, c)

                # ---------------- phase 1: QKV projections ----------------
                # K^T[m, k] for my 1024 columns; staged per-m and sent to the
                # pair AllGather. The first 3 m's six accumulation groups are
                # emitted c-major across all six PSUM buffers: while the input
                # chunks stream in, the PE issues every group's c-th matmul as
                # chunk c lands instead of head-of-line blocking on group 0's
                # final c7 matmul.
                warm = [(m, kf) for m in range(3) for kf in range(HALF // F)]
                wps = {}
                wkst = {}
                for m in range(3):
                    wkst[m] = work.tile([P, HALF], bf, tag="kst", bufs=3,
                                        name="kst")
                for g in warm:
                    wps[g] = psum.tile([P, F], f32, tag="big", bufs=6, name="pp")
                for c in range(CD):
                    for (m, kf) in warm:
                        nc.tensor.matmul(
                            wps[(m, kf)][:],
                            wk_s[:, c, m * P:(m + 1) * P],
                            xkv_s[:, c, kf * F:(kf + 1) * F],
                            start=(c == 0), stop=(c == CD - 1),
                        )
                for (m, kf) in warm:
                    nc.vector.tensor_copy(wkst[m][:, kf * F:(kf + 1) * F],
                                          wps[(m, kf)][:])
                for m in range(3):
                    nc.sync.dma_start(out=kx_in[m], in_=wkst[m][:])
                for m in range(3, CD):
                    kst = work.tile([P, HALF], bf, tag="kst", bufs=3, name="kst")
                    for kf in range(HALF // F):
                        ps = psum.tile([P, F], f32, tag="big", bufs=6, name="pp")
                        for c in range(CD):
                            nc.tensor.matmul(
                                ps[:],
                                wk_s[:, c, m * P:(m + 1) * P],
                                xkv_s[:, c, kf * F:(kf + 1) * F],
                                start=(c == 0), stop=(c == CD - 1),
                            )
                        nc.vector.tensor_copy(kst[:, kf * F:(kf + 1) * F], ps[:])
                    nc.sync.dma_start(out=kx_in[m], in_=kst[:])
                nc.gpsimd.collective_compute(
                    "AllGather", mybir.AluOpType.bypass,
                    replica_groups=GROUPS,
                    ins=[kx_in[:].opt()], outs=[kx_out[:].opt()],
                )
                # K readback: slot par holds parity-par strips; true position
                # of strip j is col 512*j + 256*par. Blessing is emitted after
                # the Q projection so the (in-order) DVE queue reaches it only
                # once the exchange has long completed.
                for m in range(CD):
                    for par in range(2):
                        dst = KT[:, m, :, par * CHUNK:(par + 1) * CHUNK]
                        nc.sync.dma_start(out=dst, in_=kx_out[par, m])
                # V[s, n] for my 8 s-tiles, exchanged in two halves so the
                # first 8 true s-tiles land before AV of chunk 0 needs them.
                # Sender tile st' = 2j+t of slot par is true s-tile 4j+2par+t.
                for half in range(2):
                    for st in range(4 * half, 4 * half + 4):
                        vst = work.tile([P, D], bf, tag="vst", bufs=2, name="vst")
                        for nf in range(D // F):
                            ps = psum.tile([P, F], f32, tag="big", bufs=6,
                                           name="pv")
                            for c in range(CD):
                                nc.tensor.matmul(
                                    ps[:],
                                    xkv_s[:, c, st * P:(st + 1) * P],
                                    wv_s[:, c, nf * F:(nf + 1) * F],
                                    start=(c == 0), stop=(c == CD - 1),
                                )
                            nc.vector.tensor_copy(vst[:, nf * F:(nf + 1) * F],
                                                  ps[:])
                        nc.scalar.dma_start(out=v_in[st], in_=vst[:])
                    nc.gpsimd.collective_compute(
                        "AllGather", mybir.AluOpType.bypass,
                        replica_groups=GROUPS,
                        ins=[v_in[4 * half:4 * half + 4].opt()],
                        outs=[v_out[half][:].opt()],
                    )
                    for par in range(2):
                        for stp in range(4):
                            j, t = (stp + 4 * half) // 2, stp % 2
                            dst = Vt[:, j, 2 * par + t, :]
                            nc.sync.dma_start(out=dst, in_=v_out[half][par, stp])
                # Q^T[m, q], with the K/V readback blesses interleaved into
                # the loop: DVE clears them between Q-projection drains (the
                # exchange is done by then), so the first scores group isn't
                # delayed by a blessing backlog after Q finishes.
                for m in range(CD):
                    for qf in range(HALF // F):
                        ps = psum.tile([P, F], f32, tag="big", bufs=6, name="pq")
                        for c in range(CD):
                            nc.tensor.matmul(
                                ps[:],
                                wq_s[:, c, m * P:(m + 1) * P],
                                xq_s[:, c, qf * F:(qf + 1) * F],
                                start=(c == 0), stop=(c == CD - 1),
                            )
                        nc.vector.tensor_copy(QT[:, m, qf * F:(qf + 1) * F], ps[:])
                    for par in range(2):
                        v = KT[:, m, :, par * CHUNK:(par + 1) * CHUNK].bitcast(u32)
                        nc.vector.tensor_copy(v, v)
                    v = Vt[:, m // 4, m % 4, :].bitcast(u32)
                    nc.vector.tensor_copy(v, v)

                # ---------------- phase 2: attention ----------------
                # largest chunk first: its long score/exp pipeline fills the
                # attention warm-up latency; the smallest chunk's scores are
                # emitted before the previous chunk's AV so its exp latency
                # hides under real PE work and the tail is minimal.
                vblessed = 8  # Vt tiles 0-7 blessed in the Q-proj loop above

                def scores(ci, pt_tag):
                    nonlocal vblessed
                    nk = KV[ci] // P
                    qb = QBASE[ci]
                    md = m_d[ci]
                    # bless V tiles not yet blessed (AV reads tiles [0, nk))
                    for tt in range(min(vblessed, nk), nk):
                        v = Vt[:, tt // 4, tt % 4, :].bitcast(u32)
                        nc.vector.tensor_copy(v, v)
                    vblessed = max(vblessed, nk)
                    # P^T = exp((S^T + mask)/32), bf16, reuses a dead slot
                    PT = work.tile([P, SK, CHUNK], bf, tag=pt_tag, name="PT")
                    vmin = min(QROWS[0][ci], QROWS[1][ci])
                    # k-tiles processed in pairs sharing one [128,512] PSUM
                    # bank: one mask-add / exp / P^T-copy per pair halves the
                    # drain-chain op count and doubles the k-tiles in flight.
                    for kg in range(nk // 2):
                        masked = (2 * kg + 1) * P + P - 1 > vmin
                        if masked:
                            mt = work.tile([P, 2, CHUNK], bf, tag="mask",
                                           bufs=4, name="mt")
                            for j in range(2):
                                ki = 2 * kg + j
                                nc.scalar.dma_start(
                                    out=mt[:, j], in_=md[ki * P:(ki + 1) * P, :])
                            mv = mt[:].bitcast(u32)
                            nc.vector.tensor_copy(mv, mv)
                        ps = psum.tile([P, 2 * CHUNK], f32, tag="big", bufs=6,
                                       name="psc")
                        for j in range(2):
                            ki = 2 * kg + j
                            for c in range(CD):
                                nc.tensor.matmul(
                                    ps[:, j * CHUNK:(j + 1) * CHUNK],
                                    KT[:, c, ki // 4, (ki % 4) * P:(ki % 4 + 1) * P],
                                    QT[:, c, qb:qb + CHUNK],
                                    start=(c == 0), stop=(c == CD - 1),
                                )
                        if masked:
                            nc.vector.tensor_add(ps[:], ps[:], mt[:])
                        pe = work.tile([P, 2 * CHUNK], bf, tag="pexp", bufs=2,
                                       name="pe")
                        nc.scalar.activation(
                            pe[:], ps[:],
                            mybir.ActivationFunctionType.Exp, scale=SCALE,
                        )
                        nc.vector.tensor_copy(PT[:, 2 * kg:2 * kg + 2], pe[:])
                    return PT

                def av(ci, PT):
                    nk = KV[ci] // P
                    qb = QBASE[ci]
                    for qj in range(CHUNK // P):
                        o0 = psum.tile([P, F], f32, tag="big", bufs=6, name="o0")
                        o1 = psum.tile([P, F], f32, tag="big", bufs=6, name="o1")
                        rs = psum.tile([P, 1], f32, tag="rs", bufs=2, name="rs")
                        for ki in range(nk):
                            lh = PT[:, ki, qj * P:(qj + 1) * P]
                            st_, sp_ = (ki == 0), (ki == nk - 1)
                            nc.tensor.matmul(o0[:], lh,
                                             Vt[:, ki // 4, ki % 4, 0:F],
                                             start=st_, stop=sp_)
                            nc.tensor.matmul(o1[:], lh,
                                             Vt[:, ki // 4, ki % 4, F:2 * F],
                                             start=st_, stop=sp_)
                            nc.tensor.matmul(rs[:], lh, ones[:, 0:1],
                                             start=st_, stop=sp_)
                        rcp = work.tile([P, 1], f32, tag="rcp", bufs=4,
                                        name="rcp")
                        nc.vector.reciprocal(rcp[:], rs[:])
                        ot = work.tile([P, D], bf, tag="ot", bufs=4, name="ot")
                        row = qb + qj * P
                        nc.vector.tensor_scalar_mul(ot[:, 0:F], o0[:], rcp[:])
                        nc.sync.dma_start(out=out_d[row:row + P, 0:F],
                                          in_=ot[:, 0:F])
                        nc.vector.tensor_scalar_mul(ot[:, F:2 * F], o1[:], rcp[:])
                        nc.sync.dma_start(out=out_d[row:row + P, F:2 * F],
                                          in_=ot[:, F:2 * F])

                # chunk order: 1 (warm-up, V tiles 0-7 = first V exchange
                # only), 2, 3 (need the second V exchange progressively),
                # 0 last (its scores are emitted under av(3) so the tiny
                # tail is just av(0)).
                pt1 = scores(1, "xkv")
                av(1, pt1)
                pt2 = scores(2, "xkv")
                av(2, pt2)
                pt3 = scores(3, "xkv")
                pt0 = scores(0, "xq")
                av(3, pt3)
                av(0, pt0)
    nc.finalize()  # run bacc legalization (wait splitting, reg alloc)
    return nc


_NC_CACHE = {}


def _get_nc(reps: int = 1):
    if reps not in _NC_CACHE:
        _NC_CACHE[reps] = build_nc(reps)
    return _NC_CACHE[reps]


def _masks():
    """Additive bf16 masks per half: 0 where k <= global q position, else -1e30."""
    q = np.arange(CHUNK)[None, :]
    out = []
    for h in range(2):
        ms = []
        for ci in range(len(KV)):
            k = np.arange(KV[ci])[:, None]
            ms.append(np.where(k <= q + QROWS[h][ci], 0.0, NEG).astype(BF16))
        out.append(ms)
    return out


def make_in_maps(x, Wq, Wk, Wv):
    wqb = np.ascontiguousarray(np.asarray(Wq).astype(BF16))
    wkb = np.ascontiguousarray(np.asarray(Wk).astype(BF16))
    wvb = np.ascontiguousarray(np.asarray(Wv).astype(BF16))
    masks = _masks()
    in_maps = []
    xT = [np.ascontiguousarray(np.asarray(x)[b].T.astype(BF16)) for b in range(B)]
    for i in range(8):
        b, h = i // 2, i % 2
        # my parity's 256-col blocks packed into [0:1024)
        xkv = np.ascontiguousarray(np.concatenate(
            [xT[b][:, 512 * j + 256 * h: 512 * j + 256 * h + 256]
             for j in range(4)], axis=1))
        xq = np.ascontiguousarray(np.concatenate(
            [xT[b][:, r:r + CHUNK] for r in QROWS[h]], axis=1))
        m = {"xkv": xkv, "xq": xq, "wq": wqb, "wk": wkb, "wv": wvb}
        for ci in range(len(KV)):
            m[f"mask{ci}"] = masks[h][ci]
        in_maps.append(m)
    return in_maps


def gather_out(results, x_dtype=np.float32):
    out = np.empty((B, S, D), x_dtype)
    for i in range(8):
        b, h = i // 2, i % 2
        o = np.asarray(results[i]["out"]).astype(x_dtype)
        for ci, r in enumerate(QROWS[h]):
            out[b, r:r + CHUNK] = o[ci * CHUNK:(ci + 1) * CHUNK]
    return out


def run_cores(in_maps, **kwargs):
    return run_bass_kernel_spmd(_get_nc(), in_maps, core_ids=list(range(8)), **kwargs)


def kernel(x, Wq, Wk, Wv):
    x = np.asarray(x)
    in_maps = make_in_maps(x, np.asarray(Wq), np.asarray(Wk), np.asarray(Wv))
    res = run_cores(in_maps)
    return gather_out(res.results)
